# revision 29
# baseline (speedup 1.0000x reference)
"""Trainium2 Bass kernel for nn_Attention_Module (sparse_attention).

Computation per batch b (x_b: [C=256, T=4096] fp32):
    energy = x_b @ x_b^T                      # (256, 256), K=4096
    attn   = softmax(rowmax(energy) - energy) # == exp(mu - e)/Z, mu = rowmin
    out    = gamma * (attn @ x_b) + x_b

Strategy (8 cores, pure data-parallel, 4 batches/core):
  - Host pre-swizzles both x layouts so every DMA transfer is a fat
    contiguous run (16-32 KB per partition): 1 KB-granular descriptors
    previously made HWDGE dispatch the bottleneck (~10 us per 2 MB load).
  - xt (t-on-partition, fp32) feeds the energy matmul as float32r;
    xn (c-on-partition, fp16) feeds the second matmul and the residual.
  - The +x residual is folded into matmul2 via the modified attention matrix
    A'' = gamma*P^T + diag(Z); out = diag(1/Z) * (A''^T @ x).
  - Output is stored fp16 (tolerance is 2e-2); host upcasts to fp32.
"""

import numpy as np

B, C, T = 32, 256, 4096
NCORES = 8
NB = B // NCORES  # batches per core
P = 128
KT = T // P  # 32 k-tiles for the energy matmul
TC = T // 512  # 8 t-chunks for matmul2

_CACHE = {}


def _build_nc(variant=None):
    variant = variant or {}
    from contextlib import ExitStack

    import concourse.bacc as bacc
    import concourse.bass as bass
    import concourse.tile as tile
    from concourse import mybir

    f32 = mybir.dt.float32
    f32r = mybir.dt.float32r
    f16 = mybir.dt.float16
    ts = bass.ts

    nc = bacc.Bacc(
        "TRN2",
        target_bir_lowering=False,
        debug=False,
        enable_asserts=False,
        num_devices=NCORES,
    )

    # host-swizzled layouts: contiguous fat runs per partition
    xt_h = nc.dram_tensor("xt", [NB, P, KT * C], f32r, kind="ExternalInput")
    xn_h = nc.dram_tensor("xn", [NB, P, 2 * T], f16, kind="ExternalInput")
    # aux: per-partition row [gamma, pad, pad, pad, identity-row(128)]
    aux_h = nc.dram_tensor("aux", [P, 132], f32, kind="ExternalInput")
    o_h = nc.dram_tensor("o", [NB, P, 2 * T], f16, kind="ExternalOutput")

    with tile.TileContext(nc) as tc:
        with ExitStack() as ctx:
            singles = ctx.enter_context(tc.tile_pool(name="singles", bufs=1))
            xt_pool = ctx.enter_context(tc.tile_pool(name="xt", bufs=2))
            xq_pool = ctx.enter_context(tc.tile_pool(name="xq", bufs=1))
            xn_pool = ctx.enter_context(tc.tile_pool(name="xn", bufs=3))
            out_pool = ctx.enter_context(tc.tile_pool(name="out", bufs=2))
            att_pool = ctx.enter_context(tc.tile_pool(name="att", bufs=3))
            small = ctx.enter_context(tc.tile_pool(name="small", bufs=4))
            psum_e = ctx.enter_context(
                tc.tile_pool(name="psum_e", bufs=2, space="PSUM")
            )
            psum_t = ctx.enter_context(
                tc.tile_pool(name="psum_t", bufs=2, space="PSUM")
            )
            psum_o = ctx.enter_context(
                tc.tile_pool(name="psum_o", bufs=4, space="PSUM")
            )

            xt_ap = xt_h.ap()
            xn_ap = xn_h.ap()
            o_ap = o_h.ap()

            # aux on the ACT ring so it doesn't delay the first xt load
            aux = singles.tile([P, 132], f32)
            nc.scalar.dma_start(aux[:], aux_h.ap())
            gv = aux[:, 0:1]
            ident = aux[:, 4:132]

            KH = KT // 2

            B0_SPANS = [(0, 2), (2, 2), (4, 4), (8, 8), (16, 8), (24, 8)]

            def issue_loads(b):
                if b == 0:
                    # fine-grained tiles so matmul1 starts after the first
                    # 0.25 MB lands (Tile tracks deps at tile granularity)
                    tls = []
                    for i, (k0, kn) in enumerate(B0_SPANS):
                        t_ = xq_pool.tile(
                            [P, kn, C], f32r, tag=f"xq{i}", name=f"xq{i}"
                        )
                        nc.sync.dma_start(
                            t_[:], xt_ap[b][:, k0 * C : (k0 + kn) * C]
                        )
                        tls.append((k0, kn, t_))
                else:
                    xta = xt_pool.tile([P, KH, C], f32r, tag="xta", name="xta")
                    xtb = xt_pool.tile([P, KH, C], f32r, tag="xtb", name="xtb")
                    nc.sync.dma_start(xta[:], xt_ap[b][:, : KH * C])
                    nc.sync.dma_start(xtb[:], xt_ap[b][:, KH * C :])
                    tls = [(0, KH, xta), (KH, KH, xtb)]
                xn = xn_pool.tile([P, 2, T], f16, tag="xn", name="xn")
                nc.sync.dma_start(xn[:], xn_ap[b])
                return tls, xn

            def src_at(tls, k):
                for k0, kn, t_ in tls:
                    if k0 <= k < k0 + kn:
                        return t_, k - k0
                raise AssertionError

            tiles = {0: issue_loads(0)}
            pending = None  # (b, At, rZ, xn) awaiting matmul2

            for b in range(NB):
                tls, xn = tiles.pop(b)
                if b + 1 < NB:
                    tiles[b + 1] = issue_loads(b + 1)

                # A''^T, laid out [128(j within k-block), k-block, 256(i)]
                At = att_pool.tile([P, 2, C], f16)
                Zs = small.tile([P, 2], f32)
                Zb = small.tile([P, 2], f16)
                rZ = small.tile([P, 2], f32)

                for m in range(2):
                    pe = psum_e.tile([P, C], mybir.dt.float32)
                    for k in range(KT):
                        src_t, kk = src_at(tls, k)
                        nc.tensor.matmul(
                            pe[:],
                            lhsT=src_t[:, kk, ts(m, P)],
                            rhs=src_t[:, kk, :],
                            start=(k == 0),
                            stop=(k == KT - 1),
                        )
                    mu = small.tile([P, 1], f32)
                    nc.vector.tensor_reduce(
                        mu[:], pe[:], axis=mybir.AxisListType.X,
                        op=mybir.AluOpType.min,
                    )
                    Pm = small.tile([P, C], f32, tag="Pm")
                    nc.scalar.activation(
                        Pm[:],
                        pe[:],
                        mybir.ActivationFunctionType.Exp,
                        bias=mu[:],
                        scale=-1.0,
                        accum_out=Zs[:, m : m + 1],
                    )
                    nc.vector.tensor_copy(Zb[:, m : m + 1], Zs[:, m : m + 1])
                    nc.vector.reciprocal(rZ[:, m : m + 1], Zb[:, m : m + 1])
                    for k in range(2):
                        pt = psum_t.tile([P, P], mybir.dt.float32)
                        nc.tensor.transpose(pt[:], Pm[:, ts(k, P)], ident)
                        # A''T[j in k-block, i in m-block] = gamma * P^T
                        nc.scalar.mul(At[:, k, ts(m, P)], pt[:], gv)
                    # diagonal: += diag(Z) (falls in the k == m block)
                    dg = small.tile([P, P], f16, tag="diag")
                    nc.vector.tensor_scalar_mul(dg[:], ident, Zs[:, m : m + 1])
                    nc.vector.tensor_add(
                        At[:, m, ts(m, P)], At[:, m, ts(m, P)], dg[:]
                    )

                # software-pipeline the PE: run the PREVIOUS batch's matmul2
                # after this batch's matmul1, hiding the A'' build latency.
                this = (b, At, rZ, xn)
                todo = [pending] if pending is not None else []
                if b == NB - 1:
                    todo.append(this)
                    pending = None
                else:
                    pending = this
                for pb, pAt, prZ, pxn in todo:
                    last = pb == NB - 1
                    for m in range(2):
                        if last:
                            # separate per-group tiles: stores fire as soon
                            # as each 0.5 MB chunk's scales complete
                            ots = [
                                out_pool.tile(
                                    [P, 1024], f16, tag=f"otl{c}",
                                    name=f"otl{c}",
                                )
                                for c in range(4)
                            ]
                        else:
                            ot = out_pool.tile(
                                [P, T], f16, tag="ot", name="ot"
                            )
                        for g in range(TC // 2):
                            pos = [
                                psum_o.tile(
                                    [P, 512], mybir.dt.float32,
                                    name="po", tag="po",
                                )
                                for j in range(2)
                            ]
                            # same-weight runs of 2: walrus ldw-opt elides
                            # the redundant weight reload
                            for k in range(2):
                                for j in range(2):
                                    nc.tensor.matmul(
                                        pos[j][:],
                                        lhsT=pAt[:, k, ts(m, P)],
                                        rhs=pxn[:, k, ts(2 * g + j, 512)],
                                        start=(k == 0),
                                        stop=(k == 1),
                                    )
                            for j in range(2):
                                t8 = 2 * g + j
                                dst = (
                                    ots[g][:, ts(j, 512)]
                                    if last
                                    else ot[:, ts(t8, 512)]
                                )
                                if t8 % 2 == 0:
                                    nc.vector.tensor_scalar_mul(
                                        dst, pos[j][:], prZ[:, m : m + 1]
                                    )
                                else:
                                    nc.scalar.mul(
                                        dst, pos[j][:], prZ[:, m : m + 1]
                                    )
                            if last:
                                nc.sync.dma_start(
                                    o_ap[pb][:, m * T :][:, ts(g, 1024)],
                                    ots[g][:],
                                )
                        if not last:
                            for sh in range(2):
                                nc.sync.dma_start(
                                    o_ap[pb][:, m * T :][:, ts(sh, T // 2)],
                                    ot[:, ts(sh, T // 2)],
                                )

    nc.compile()
    return nc


def _get_nc():
    if "nc" not in _CACHE:
        _CACHE["nc"] = _build_nc()
    return _CACHE["nc"]


def _make_aux(gamma_val):
    aux = np.zeros((P, 132), dtype=np.float32)
    aux[:, 0] = gamma_val
    aux[:, 4:132] = np.eye(P, dtype=np.float32)
    return aux


def kernel(x, gamma, _trace=False):
    import concourse.bass_utils as bass_utils

    x = np.ascontiguousarray(np.asarray(x, dtype=np.float32))
    gamma = np.asarray(gamma, dtype=np.float32).reshape(-1)

    nc = _get_nc()

    aux = _make_aux(gamma[0])
    in_maps = []
    for d in range(NCORES):
        xs = x[d * NB : (d + 1) * NB]
        # xt[b, p, k*C+c] = x[b, c, k*128+p]  (fat contiguous runs)
        xt = np.ascontiguousarray(
            xs.transpose(0, 2, 1)
            .reshape(NB, KT, P, C)
            .transpose(0, 2, 1, 3)
            .reshape(NB, P, KT * C)
        )
        # xn[b, p, m*T+t] = x[b, m*128+p, t]
        xn = np.ascontiguousarray(
            xs.reshape(NB, 2, P, T).transpose(0, 2, 1, 3).reshape(NB, P, 2 * T)
        ).astype(np.float16)
        in_maps.append({"xt": xt, "xn": xn, "aux": aux})

    res = bass_utils.run_bass_kernel_spmd(
        nc, in_maps, core_ids=list(range(NCORES)), trace=_trace
    )
    # o[b, p, m*T+t] = out[b, m*128+p, t]
    out = np.concatenate(
        [
            r["o"].reshape(NB, P, 2, T).transpose(0, 2, 1, 3).reshape(NB, C, T)
            for r in res.results
        ],
        axis=0,
    ).astype(np.float32)
    if _trace:
        _CACHE["last_results"] = res
    return out


# revision 31
# speedup vs baseline: 1.0043x; 1.0043x over previous
"""Trainium2 Bass kernel for nn_Attention_Module (sparse_attention).

Computation per batch b (x_b: [C=256, T=4096] fp32):
    energy = x_b @ x_b^T                      # (256, 256), K=4096
    attn   = softmax(rowmax(energy) - energy) # == exp(mu - e)/Z, mu = rowmin
    out    = gamma * (attn @ x_b) + x_b

Strategy (8 cores, pure data-parallel, 4 batches/core):
  - Host pre-swizzles both x layouts so every DMA transfer is a fat
    contiguous run (16-32 KB per partition): 1 KB-granular descriptors
    previously made HWDGE dispatch the bottleneck (~10 us per 2 MB load).
  - xt (t-on-partition, fp32) feeds the energy matmul as float32r;
    xn (c-on-partition, fp16) feeds the second matmul and the residual.
  - The +x residual is folded into matmul2 via the modified attention matrix
    A'' = gamma*P^T + diag(Z); out = diag(1/Z) * (A''^T @ x).
  - Output is stored fp16 (tolerance is 2e-2); host upcasts to fp32.
"""

import numpy as np

B, C, T = 32, 256, 4096
NCORES = 8
NB = B // NCORES  # batches per core
P = 128
KT = T // P  # 32 k-tiles for the energy matmul
TC = T // 512  # 8 t-chunks for matmul2

_CACHE = {}


def _build_nc(variant=None):
    variant = variant or {}
    from contextlib import ExitStack

    import concourse.bacc as bacc
    import concourse.bass as bass
    import concourse.tile as tile
    from concourse import mybir

    f32 = mybir.dt.float32
    f32r = mybir.dt.float32r
    f16 = mybir.dt.float16
    ts = bass.ts

    nc = bacc.Bacc(
        "TRN2",
        target_bir_lowering=False,
        debug=False,
        enable_asserts=False,
        num_devices=NCORES,
    )

    # host-swizzled layouts: contiguous fat runs per partition
    xt_h = nc.dram_tensor("xt", [NB, P, KT * C], f32r, kind="ExternalInput")
    xn_h = nc.dram_tensor("xn", [NB, P, 2 * T], f16, kind="ExternalInput")
    # aux: per-partition row [gamma, pad, pad, pad, identity-row(128)]
    aux_h = nc.dram_tensor("aux", [P, 132], f32, kind="ExternalInput")
    o_h = nc.dram_tensor("o", [NB, P, 2 * T], f16, kind="ExternalOutput")

    with tile.TileContext(nc) as tc:
        with ExitStack() as ctx:
            singles = ctx.enter_context(tc.tile_pool(name="singles", bufs=1))
            xt_pool = ctx.enter_context(tc.tile_pool(name="xt", bufs=2))
            xq_pool = ctx.enter_context(tc.tile_pool(name="xq", bufs=1))
            xn_pool = ctx.enter_context(tc.tile_pool(name="xn", bufs=3))
            out_pool = ctx.enter_context(tc.tile_pool(name="out", bufs=2))
            att_pool = ctx.enter_context(tc.tile_pool(name="att", bufs=3))
            small = ctx.enter_context(tc.tile_pool(name="small", bufs=4))
            psum_e = ctx.enter_context(
                tc.tile_pool(name="psum_e", bufs=2, space="PSUM")
            )
            psum_t = ctx.enter_context(
                tc.tile_pool(name="psum_t", bufs=1, space="PSUM")
            )
            psum_o = ctx.enter_context(
                tc.tile_pool(name="psum_o", bufs=1, space="PSUM")
            )

            xt_ap = xt_h.ap()
            xn_ap = xn_h.ap()
            o_ap = o_h.ap()

            # aux on the ACT ring so it doesn't delay the first xt load
            aux = singles.tile([P, 132], f32)
            nc.scalar.dma_start(aux[:], aux_h.ap())
            gv = aux[:, 0:1]
            ident = aux[:, 4:132]

            KH = KT // 2

            B0_SPANS = [(0, 2), (2, 2), (4, 4), (8, 8), (16, 8), (24, 8)]

            def issue_loads(b):
                if b == 0:
                    # fine-grained tiles so matmul1 starts after the first
                    # 0.25 MB lands (Tile tracks deps at tile granularity)
                    tls = []
                    for i, (k0, kn) in enumerate(B0_SPANS):
                        t_ = xq_pool.tile(
                            [P, kn, C], f32r, tag=f"xq{i}", name=f"xq{i}"
                        )
                        nc.sync.dma_start(
                            t_[:], xt_ap[b][:, k0 * C : (k0 + kn) * C]
                        )
                        tls.append((k0, kn, t_))
                else:
                    xta = xt_pool.tile([P, KH, C], f32r, tag="xta", name="xta")
                    xtb = xt_pool.tile([P, KH, C], f32r, tag="xtb", name="xtb")
                    nc.sync.dma_start(xta[:], xt_ap[b][:, : KH * C])
                    nc.sync.dma_start(xtb[:], xt_ap[b][:, KH * C :])
                    tls = [(0, KH, xta), (KH, KH, xtb)]
                xn = xn_pool.tile([P, 2, T], f16, tag="xn", name="xn")
                nc.sync.dma_start(xn[:], xn_ap[b])
                return tls, xn

            def src_at(tls, k):
                for k0, kn, t_ in tls:
                    if k0 <= k < k0 + kn:
                        return t_, k - k0
                raise AssertionError

            tiles = {0: issue_loads(0)}
            pending = None  # (b, At, rZ, xn) awaiting matmul2

            # preallocated PSUM rings (fewer tile instances -> fewer
            # teardown semaphore clears)
            po_ring = [
                psum_o.tile([P, 512], mybir.dt.float32, name=f"po{i}")
                for i in range(4)
            ]
            pt_ring = [
                psum_t.tile([P, P], mybir.dt.float32, name=f"pt{i}")
                for i in range(2)
            ]
            po_i = [0]
            pt_i = [0]

            for b in range(NB):
                tls, xn = tiles.pop(b)
                if b + 1 < NB:
                    tiles[b + 1] = issue_loads(b + 1)

                # A''^T, laid out [128(j within k-block), k-block, 256(i)]
                At = att_pool.tile([P, 2, C], f16)
                Zs = small.tile([P, 2], f32)
                Zb = small.tile([P, 2], f16)
                rZ = small.tile([P, 2], f32)

                for m in range(2):
                    pe = psum_e.tile([P, C], mybir.dt.float32)
                    for k in range(KT):
                        src_t, kk = src_at(tls, k)
                        nc.tensor.matmul(
                            pe[:],
                            lhsT=src_t[:, kk, ts(m, P)],
                            rhs=src_t[:, kk, :],
                            start=(k == 0),
                            stop=(k == KT - 1),
                        )
                    mu = small.tile([P, 1], f32)
                    nc.vector.tensor_reduce(
                        mu[:], pe[:], axis=mybir.AxisListType.X,
                        op=mybir.AluOpType.min,
                    )
                    Pm = small.tile([P, C], f32, tag="Pm")
                    nc.scalar.activation(
                        Pm[:],
                        pe[:],
                        mybir.ActivationFunctionType.Exp,
                        bias=mu[:],
                        scale=-1.0,
                        accum_out=Zs[:, m : m + 1],
                    )
                    nc.vector.tensor_copy(Zb[:, m : m + 1], Zs[:, m : m + 1])
                    nc.vector.reciprocal(rZ[:, m : m + 1], Zb[:, m : m + 1])
                    for k in range(2):
                        pt = pt_ring[pt_i[0] % 2]
                        pt_i[0] += 1
                        nc.tensor.transpose(pt[:], Pm[:, ts(k, P)], ident)
                        # A''T[j in k-block, i in m-block] = gamma * P^T
                        nc.scalar.mul(At[:, k, ts(m, P)], pt[:], gv)
                    # diagonal: += diag(Z) (falls in the k == m block)
                    dg = small.tile([P, P], f16, tag="diag")
                    nc.vector.tensor_scalar_mul(dg[:], ident, Zs[:, m : m + 1])
                    nc.vector.tensor_add(
                        At[:, m, ts(m, P)], At[:, m, ts(m, P)], dg[:]
                    )

                # software-pipeline the PE: run the PREVIOUS batch's matmul2
                # after this batch's matmul1, hiding the A'' build latency.
                this = (b, At, rZ, xn)
                todo = [pending] if pending is not None else []
                if b == NB - 1:
                    todo.append(this)
                    pending = None
                else:
                    pending = this
                for pb, pAt, prZ, pxn in todo:
                    last = pb == NB - 1
                    for m in range(2):
                        if last:
                            # separate per-group tiles: stores fire as soon
                            # as each 0.5 MB chunk's scales complete
                            ots = [
                                out_pool.tile(
                                    [P, 1024], f16, tag=f"otl{c}",
                                    name=f"otl{c}",
                                )
                                for c in range(4)
                            ]
                        else:
                            ot = out_pool.tile(
                                [P, T], f16, tag="ot", name="ot"
                            )
                        for g in range(TC // 2):
                            pos = [
                                po_ring[(po_i[0] + j) % 4] for j in range(2)
                            ]
                            po_i[0] += 2
                            # same-weight runs of 2: walrus ldw-opt elides
                            # the redundant weight reload
                            for k in range(2):
                                for j in range(2):
                                    nc.tensor.matmul(
                                        pos[j][:],
                                        lhsT=pAt[:, k, ts(m, P)],
                                        rhs=pxn[:, k, ts(2 * g + j, 512)],
                                        start=(k == 0),
                                        stop=(k == 1),
                                    )
                            for j in range(2):
                                t8 = 2 * g + j
                                dst = (
                                    ots[g][:, ts(j, 512)]
                                    if last
                                    else ot[:, ts(t8, 512)]
                                )
                                if t8 % 2 == 0:
                                    nc.vector.tensor_scalar_mul(
                                        dst, pos[j][:], prZ[:, m : m + 1]
                                    )
                                else:
                                    nc.scalar.mul(
                                        dst, pos[j][:], prZ[:, m : m + 1]
                                    )
                            if last:
                                nc.sync.dma_start(
                                    o_ap[pb][:, m * T :][:, ts(g, 1024)],
                                    ots[g][:],
                                )
                        if not last:
                            for sh in range(2):
                                nc.sync.dma_start(
                                    o_ap[pb][:, m * T :][:, ts(sh, T // 2)],
                                    ot[:, ts(sh, T // 2)],
                                )

    nc.compile()
    return nc


def _get_nc():
    if "nc" not in _CACHE:
        _CACHE["nc"] = _build_nc()
    return _CACHE["nc"]


def _make_aux(gamma_val):
    aux = np.zeros((P, 132), dtype=np.float32)
    aux[:, 0] = gamma_val
    aux[:, 4:132] = np.eye(P, dtype=np.float32)
    return aux


def kernel(x, gamma, _trace=False):
    import concourse.bass_utils as bass_utils

    x = np.ascontiguousarray(np.asarray(x, dtype=np.float32))
    gamma = np.asarray(gamma, dtype=np.float32).reshape(-1)

    nc = _get_nc()

    aux = _make_aux(gamma[0])
    in_maps = []
    for d in range(NCORES):
        xs = x[d * NB : (d + 1) * NB]
        # xt[b, p, k*C+c] = x[b, c, k*128+p]  (fat contiguous runs)
        xt = np.ascontiguousarray(
            xs.transpose(0, 2, 1)
            .reshape(NB, KT, P, C)
            .transpose(0, 2, 1, 3)
            .reshape(NB, P, KT * C)
        )
        # xn[b, p, m*T+t] = x[b, m*128+p, t]
        xn = np.ascontiguousarray(
            xs.reshape(NB, 2, P, T).transpose(0, 2, 1, 3).reshape(NB, P, 2 * T)
        ).astype(np.float16)
        in_maps.append({"xt": xt, "xn": xn, "aux": aux})

    res = bass_utils.run_bass_kernel_spmd(
        nc, in_maps, core_ids=list(range(NCORES)), trace=_trace
    )
    # o[b, p, m*T+t] = out[b, m*128+p, t]
    out = np.concatenate(
        [
            r["o"].reshape(NB, P, 2, T).transpose(0, 2, 1, 3).reshape(NB, C, T)
            for r in res.results
        ],
        axis=0,
    ).astype(np.float32)
    if _trace:
        _CACHE["last_results"] = res
    return out


# revision 32
# speedup vs baseline: 1.0054x; 1.0011x over previous
"""Trainium2 Bass kernel for nn_Attention_Module (sparse_attention).

Computation per batch b (x_b: [C=256, T=4096] fp32):
    energy = x_b @ x_b^T                      # (256, 256), K=4096
    attn   = softmax(rowmax(energy) - energy) # == exp(mu - e)/Z, mu = rowmin
    out    = gamma * (attn @ x_b) + x_b

Strategy (8 cores, pure data-parallel, 4 batches/core):
  - Host pre-swizzles both x layouts so every DMA transfer is a fat
    contiguous run (16-32 KB per partition): 1 KB-granular descriptors
    previously made HWDGE dispatch the bottleneck (~10 us per 2 MB load).
  - xt (t-on-partition, fp32) feeds the energy matmul as float32r;
    xn (c-on-partition, fp16) feeds the second matmul and the residual.
  - The +x residual is folded into matmul2 via the modified attention matrix
    A'' = gamma*P^T + diag(Z); out = diag(1/Z) * (A''^T @ x).
  - Output is stored fp16 (tolerance is 2e-2); host upcasts to fp32.
"""

import numpy as np

B, C, T = 32, 256, 4096
NCORES = 8
NB = B // NCORES  # batches per core
P = 128
KT = T // P  # 32 k-tiles for the energy matmul
TC = T // 512  # 8 t-chunks for matmul2

_CACHE = {}


def _build_nc(variant=None):
    variant = variant or {}
    from contextlib import ExitStack

    import concourse.bacc as bacc
    import concourse.bass as bass
    import concourse.tile as tile
    from concourse import mybir

    f32 = mybir.dt.float32
    f32r = mybir.dt.float32r
    f16 = mybir.dt.float16
    ts = bass.ts

    nc = bacc.Bacc(
        "TRN2",
        target_bir_lowering=False,
        debug=False,
        enable_asserts=False,
        num_devices=NCORES,
    )

    # host-swizzled layouts: contiguous fat runs per partition
    xt_h = nc.dram_tensor("xt", [NB, P, KT * C], f32r, kind="ExternalInput")
    xn_h = nc.dram_tensor("xn", [NB, P, 2 * T], f16, kind="ExternalInput")
    # aux: per-partition row [gamma, pad, pad, pad, identity-row(128)]
    aux_h = nc.dram_tensor("aux", [P, 132], f32, kind="ExternalInput")
    o_h = nc.dram_tensor("o", [NB, P, 2 * T], f16, kind="ExternalOutput")

    with tile.TileContext(nc) as tc:
        with ExitStack() as ctx:
            singles = ctx.enter_context(tc.tile_pool(name="singles", bufs=1))
            xt_pool = ctx.enter_context(tc.tile_pool(name="xt", bufs=2))
            xq_pool = ctx.enter_context(tc.tile_pool(name="xq", bufs=1))
            xn_pool = ctx.enter_context(tc.tile_pool(name="xn", bufs=3))
            out_pool = ctx.enter_context(tc.tile_pool(name="out", bufs=2))
            att_pool = ctx.enter_context(tc.tile_pool(name="att", bufs=3))
            small = ctx.enter_context(tc.tile_pool(name="small", bufs=4))
            psum_e = ctx.enter_context(
                tc.tile_pool(name="psum_e", bufs=2, space="PSUM")
            )
            psum_t = ctx.enter_context(
                tc.tile_pool(name="psum_t", bufs=1, space="PSUM")
            )
            psum_o = ctx.enter_context(
                tc.tile_pool(name="psum_o", bufs=1, space="PSUM")
            )

            xt_ap = xt_h.ap()
            xn_ap = xn_h.ap()
            o_ap = o_h.ap()

            # aux on the ACT ring so it doesn't delay the first xt load
            aux = singles.tile([P, 132], f32)
            nc.scalar.dma_start(aux[:], aux_h.ap())
            gv = aux[:, 0:1]
            ident = aux[:, 4:132]

            KH = KT // 2

            B0_SPANS = [(0, 2), (2, 2), (4, 4), (8, 8), (16, 8), (24, 8)]

            def issue_loads(b):
                if b == 0:
                    # fine-grained tiles so matmul1 starts after the first
                    # 0.25 MB lands (Tile tracks deps at tile granularity)
                    tls = []
                    for i, (k0, kn) in enumerate(B0_SPANS):
                        t_ = xq_pool.tile(
                            [P, kn, C], f32r, tag=f"xq{i}", name=f"xq{i}"
                        )
                        nc.sync.dma_start(
                            t_[:], xt_ap[b][:, k0 * C : (k0 + kn) * C]
                        )
                        tls.append((k0, kn, t_))
                else:
                    xta = xt_pool.tile([P, KH, C], f32r, tag="xta", name="xta")
                    xtb = xt_pool.tile([P, KH, C], f32r, tag="xtb", name="xtb")
                    nc.sync.dma_start(xta[:], xt_ap[b][:, : KH * C])
                    nc.sync.dma_start(xtb[:], xt_ap[b][:, KH * C :])
                    tls = [(0, KH, xta), (KH, KH, xtb)]
                xn = xn_pool.tile([P, 2, T], f16, tag="xn", name="xn")
                nc.sync.dma_start(xn[:], xn_ap[b])
                return tls, xn

            def src_at(tls, k):
                for k0, kn, t_ in tls:
                    if k0 <= k < k0 + kn:
                        return t_, k - k0
                raise AssertionError

            tiles = {0: issue_loads(0)}
            pending = None  # (b, At, rZ, xn) awaiting matmul2

            # preallocated PSUM rings (fewer tile instances -> fewer
            # teardown semaphore clears)
            po_ring = [
                psum_o.tile([P, 512], mybir.dt.float32, name=f"po{i}")
                for i in range(4)
            ]
            pt_ring = [
                psum_t.tile([P, P], mybir.dt.float32, name=f"pt{i}")
                for i in range(2)
            ]
            po_i = [0]
            pt_i = [0]

            def emit_mm2(job):
                pb, pAt, prZ, pxn = job
                last = pb == NB - 1
                for m in range(2):
                    if last:
                        ots = [
                            out_pool.tile(
                                [P, 1024], f16, tag=f"otl{c}", name=f"otl{c}"
                            )
                            for c in range(4)
                        ]
                    else:
                        ot = out_pool.tile([P, T], f16, tag="ot", name="ot")
                    for g in range(TC // 2):
                        pos = [
                            po_ring[(po_i[0] + j) % 4] for j in range(2)
                        ]
                        po_i[0] += 2
                        for k in range(2):
                            for j in range(2):
                                nc.tensor.matmul(
                                    pos[j][:],
                                    lhsT=pAt[:, k, ts(m, P)],
                                    rhs=pxn[:, k, ts(2 * g + j, 512)],
                                    start=(k == 0),
                                    stop=(k == 1),
                                )
                        for j in range(2):
                            t8 = 2 * g + j
                            dst = (
                                ots[g][:, ts(j, 512)]
                                if last
                                else ot[:, ts(t8, 512)]
                            )
                            if t8 % 2 == 0:
                                nc.vector.tensor_scalar_mul(
                                    dst, pos[j][:], prZ[:, m : m + 1]
                                )
                            else:
                                nc.scalar.mul(
                                    dst, pos[j][:], prZ[:, m : m + 1]
                                )
                        if last:
                            nc.sync.dma_start(
                                o_ap[pb][:, m * T :][:, ts(g, 1024)],
                                ots[g][:],
                            )
                    if not last:
                        for sh in range(2):
                            nc.sync.dma_start(
                                o_ap[pb][:, m * T :][:, ts(sh, T // 2)],
                                ot[:, ts(sh, T // 2)],
                            )

            for b in range(NB):
                tls, xn = tiles.pop(b)
                if b + 1 < NB:
                    tiles[b + 1] = issue_loads(b + 1)

                # run the PREVIOUS batch's matmul2 FIRST: its inputs are
                # ready, giving this batch's loads ~7us more slack
                if pending is not None:
                    emit_mm2(pending)
                    pending = None

                # A''^T, laid out [128(j within k-block), k-block, 256(i)]
                At = att_pool.tile([P, 2, C], f16)
                Zs = small.tile([P, 2], f32)
                Zb = small.tile([P, 2], f16)
                rZ = small.tile([P, 2], f32)

                for m in range(2):
                    pe = psum_e.tile([P, C], mybir.dt.float32)
                    for k in range(KT):
                        src_t, kk = src_at(tls, k)
                        nc.tensor.matmul(
                            pe[:],
                            lhsT=src_t[:, kk, ts(m, P)],
                            rhs=src_t[:, kk, :],
                            start=(k == 0),
                            stop=(k == KT - 1),
                        )
                    mu = small.tile([P, 1], f32)
                    nc.vector.tensor_reduce(
                        mu[:], pe[:], axis=mybir.AxisListType.X,
                        op=mybir.AluOpType.min,
                    )
                    Pm = small.tile([P, C], f32, tag="Pm")
                    nc.scalar.activation(
                        Pm[:],
                        pe[:],
                        mybir.ActivationFunctionType.Exp,
                        bias=mu[:],
                        scale=-1.0,
                        accum_out=Zs[:, m : m + 1],
                    )
                    nc.vector.tensor_copy(Zb[:, m : m + 1], Zs[:, m : m + 1])
                    nc.vector.reciprocal(rZ[:, m : m + 1], Zb[:, m : m + 1])
                    for k in range(2):
                        pt = pt_ring[pt_i[0] % 2]
                        pt_i[0] += 1
                        nc.tensor.transpose(pt[:], Pm[:, ts(k, P)], ident)
                        # A''T[j in k-block, i in m-block] = gamma * P^T
                        nc.scalar.mul(At[:, k, ts(m, P)], pt[:], gv)
                    # diagonal: += diag(Z) (falls in the k == m block)
                    dg = small.tile([P, P], f16, tag="diag")
                    nc.vector.tensor_scalar_mul(dg[:], ident, Zs[:, m : m + 1])
                    nc.vector.tensor_add(
                        At[:, m, ts(m, P)], At[:, m, ts(m, P)], dg[:]
                    )

                # (matmul2 for this batch is emitted at the START of the
                # next iteration -- see emit_mm2 -- so matmul1 never blocks
                # the PE FIFO while its loads stream in)
                this = (b, At, rZ, xn)
                if b == NB - 1:
                    emit_mm2(this)
                else:
                    pending = this

    nc.compile()
    return nc


def _get_nc():
    if "nc" not in _CACHE:
        _CACHE["nc"] = _build_nc()
    return _CACHE["nc"]


def _make_aux(gamma_val):
    aux = np.zeros((P, 132), dtype=np.float32)
    aux[:, 0] = gamma_val
    aux[:, 4:132] = np.eye(P, dtype=np.float32)
    return aux


def kernel(x, gamma, _trace=False):
    import concourse.bass_utils as bass_utils

    x = np.ascontiguousarray(np.asarray(x, dtype=np.float32))
    gamma = np.asarray(gamma, dtype=np.float32).reshape(-1)

    nc = _get_nc()

    aux = _make_aux(gamma[0])
    in_maps = []
    for d in range(NCORES):
        xs = x[d * NB : (d + 1) * NB]
        # xt[b, p, k*C+c] = x[b, c, k*128+p]  (fat contiguous runs)
        xt = np.ascontiguousarray(
            xs.transpose(0, 2, 1)
            .reshape(NB, KT, P, C)
            .transpose(0, 2, 1, 3)
            .reshape(NB, P, KT * C)
        )
        # xn[b, p, m*T+t] = x[b, m*128+p, t]
        xn = np.ascontiguousarray(
            xs.reshape(NB, 2, P, T).transpose(0, 2, 1, 3).reshape(NB, P, 2 * T)
        ).astype(np.float16)
        in_maps.append({"xt": xt, "xn": xn, "aux": aux})

    res = bass_utils.run_bass_kernel_spmd(
        nc, in_maps, core_ids=list(range(NCORES)), trace=_trace
    )
    # o[b, p, m*T+t] = out[b, m*128+p, t]
    out = np.concatenate(
        [
            r["o"].reshape(NB, P, 2, T).transpose(0, 2, 1, 3).reshape(NB, C, T)
            for r in res.results
        ],
        axis=0,
    ).astype(np.float32)
    if _trace:
        _CACHE["last_results"] = res
    return out
